# revision 53
# baseline (speedup 1.0000x reference)
"""Trainium2 Bass kernel for nn_DecoderLayer_68212670595779.

Sharding: each of the 8 cores owns one head PAIR (heads 2k, 2k+1) for BOTH
batches, and (batch, token-slice k*256:(k+1)*256) for the FFN. Attention
output ships UNNORMALIZED through a per-batch 8-rank AllToAll with the
softmax denominators as 2 extra payload rows per shard; normalization
happens post-collective. The batch-0 collective hides under batch-1
attention; the batch-1 collective hides under the batch-0 FFN half.
"""
import sys

sys.path.insert(0, "/opt/trn_rl_repo")

import numpy as np
import ml_dtypes
from contextlib import ExitStack

import concourse.bass as bass
import concourse.mybir as mybir
import concourse.tile as tile
from concourse.vector_clock import ScopedClock
from concourse.bass_utils import run_bass_kernel_spmd

BF16 = ml_dtypes.bfloat16
FP32 = mybir.dt.float32
BF = mybir.dt.bfloat16
AF = mybir.ActivationFunctionType
ALU = mybir.AluOpType
AX = mybir.AxisListType

B, S, D, H, HD, FF, P = 2, 2048, 1024, 16, 64, 4096, 128
NCORES = 8
QT_TOK = 256  # tokens per (core, batch) in phase 2


# ---------------------------------------------------------------------------
# Workaround: this walrus build allows only ONE semaphore wait on a CTRL
# (Drain) instruction; TileContext's final drain carries one wait per busy
# proc. Split the waits across a chain of drains on the same engine.
def _patched_drain_and_barrier(self, tick_clock, wait_clock):
    nc = self.nc
    drain_inst = nc.sync.drain()
    wait_clock.add_sem_waits(
        drain_inst.ins, ScopedClock({None: tick_clock.global_clock})
    )
    si = drain_inst.ins.sync_info
    waits = list(si.on_wait) if si is not None else []
    if len(waits) > 1:
        si.on_wait = waits[:1]
        for w in waits[1:]:
            extra = nc.sync.drain()
            esi = extra.ins.sync_info
            if esi is None:
                extra.ins.sync_info = mybir.SyncInfo(on_wait=[w], on_update=[])
            else:
                esi.on_wait = [w]
    nc.all_engine_barrier()
    assert self.sems is not None
    popped = nc._tile_sem_poison_stack.pop()
    assert popped is self._sem_poison
    nc.clear_and_free_semaphores(list(self.sems.allocated().values()))
    nc.all_engine_barrier()


tile.TileContext._drain_and_barrier = _patched_drain_and_barrier


def _split_multi_waits(nc):
    """Walrus in this container supports a single sem wait per instruction.
    Move extra waits onto dedicated no-op instructions on the same engine,
    inserted immediately before (engine program order preserves semantics)."""
    n_split = 0
    for fn in nc.m.functions:
        for bb in fn.blocks:
            out = []
            for ins in bb.instructions:
                si = ins.sync_info
                waits = list(si.on_wait) if si is not None else []
                if len(waits) > 1:
                    si.on_wait = [waits[-1]]
                    for i, w in enumerate(waits[:-1]):
                        nop = mybir.InstNoOp(
                            name=f"{ins.name}-sw{i}",
                            engine=ins.engine,
                            bass_nofuse=True,
                            sync_info=mybir.SyncInfo(on_wait=[w], on_update=[]),
                        )
                        out.append(nop)
                        n_split += 1
                out.append(ins)
            bb.instructions[:] = out
    return n_split


def _tri_mask():
    """[kv 128, q 128]: -240 where kv > q (masked), else 0. Added to scores
    pre-exp: exp(0.125*(s-240)) ~ 1e-13, i.e. effectively zero weight."""
    a = np.arange(P)[:, None]
    qq = np.arange(P)[None, :]
    return np.where(a > qq, -240.0, 0.0).astype(np.float32).astype(BF16)


def _build_nc(reps=1):
    nc = bass.Bass()

    def din(name, shape, dt=BF):
        return nc.declare_dram_parameter(name, list(shape), dt, isOutput=False)

    xT_d = din("xT", (P, 2, 8, S))          # x[b].T, bf16, kc-chunked
    cb_d = din("cb", (P, 2 * P))            # packed bf16: mask | eye
    cf_d = din("cf", (P, 50), FP32)         # packed fp32: bq|bk|bv|bo|b1
    wq_d = din("wq", (P, 8, P))             # Wq[:, 2k*64:(2k+2)*64] kc-chunked
    wk_d = din("wk", (P, 8, P))
    wv_d = din("wv", (P, 8, P))
    wo_d = din("wo", (P, 8, D))
    w1_d = din("w1", (P, 8, FF))
    w2_d = din("w2", (P, 32, D))
    selm_d = din("selm", (16, 8, P))        # denom-row selection matrices
    rows_d = din("rows", (65, D), FP32)     # b2@0 / gamma@32 / beta@64
    out_d = nc.declare_dram_parameter("out", [2 * QT_TOK * 2, D], FP32, isOutput=True)
    # per-batch A2A buffers: 8 shards x (128 dim rows + 2 denom rows) x 256 tok
    a2a_in = [nc.dram_tensor(f"a2ain{b}", [8 * 130, QT_TOK], BF) for b in range(2)]
    a2a_out = [nc.dram_tensor(f"a2aout{b}", [8 * 130, QT_TOK], BF) for b in range(2)]

    with ExitStack() as top:
        tc = top.enter_context(tile.TileContext(nc))

        const = top.enter_context(tc.tile_pool(name="const", bufs=1))

        # ---- first xT chunk ahead of everything (feeds the first matmul);
        # constants packed into few DMAs (dispatch overhead matters) ----
        xp0 = top.enter_context(tc.tile_pool(name="xp0", bufs=1))
        xt00 = xp0.tile([P, 8, 512], BF, tag="xt00")
        nc.sync.dma_start(xt00[:, 0:2, :], xT_d[:, 0, 0:2, 0:512])
        nc.sync.dma_start(xt00[:, 2:8, :], xT_d[:, 0, 2:8, 0:512])
        cb_sb = const.tile([P, 2 * P], BF, tag="cb")
        nc.sync.dma_start(cb_sb[:], cb_d[:])
        cf_sb = const.tile([P, 50], FP32, tag="cf")
        nc.sync.dma_start(cf_sb[:], cf_d[:])
        ones_sb = const.tile([P, P], FP32, tag="ones")
        nc.vector.memset(ones_sb[:], 1.0)
        mask_sb = cb_sb[:, 0:P]
        eye_sb = cb_sb[:, P : 2 * P]
        bq_sb = cf_sb[:, 0:1]
        bk_sb = cf_sb[:, 1:2]
        bv_sb = cf_sb[:, 2:10]
        bo_sb = cf_sb[:, 10:18]
        b1_sb = cf_sb[:, 18:50]

        if reps > 1:
            # timing-only variant: repeat the whole body on-device so HW time
            # dominates host/tunnel dispatch overhead
            top.enter_context(tc.For_i(0, reps, 1))

        # ---- QKV weights (Act HWDGE queue, needed immediately) ----
        wbulk = top.enter_context(tc.tile_pool(name="wbulk", bufs=1))
        wq_sb = wbulk.tile([P, 8, P], BF, tag="wq")
        nc.scalar.dma_start(wq_sb[:, 0:2, :], wq_d[:, 0:2, :])
        nc.scalar.dma_start(wq_sb[:, 2:8, :], wq_d[:, 2:8, :])
        wk_sb = wbulk.tile([P, 8, P], BF, tag="wk")
        nc.scalar.dma_start(wk_sb[:], wk_d[:])
        wv_sb = wbulk.tile([P, 8, P], BF, tag="wv")
        nc.scalar.dma_start(wv_sb[:], wv_d[:])
        selm_sb = wbulk.tile([16, 8, P], BF, tag="selm", name="selm_sb")
        nc.scalar.dma_start(selm_sb[:], selm_d[:])
        rows_sb = wbulk.tile([65, D], FP32, tag="rows", name="rows_sb")
        nc.scalar.dma_start(rows_sb[:], rows_d[:])
        wo_sb = wbulk.tile([P, 8, D], BF, tag="wo")
        w1_sb = wbulk.tile([P, 8, FF], BF, tag="w1")
        # cross-phase tiles for collective receive (emitted in phase 1)
        attVf = [
            wbulk.tile([P, 8, QT_TOK], BF, tag=f"vf{b}", name=f"attVf{b}")
            for b in range(2)
        ]
        denf = [
            wbulk.tile([16, QT_TOK], BF, tag=f"den{b}", name=f"denf{b}")
            for b in range(2)
        ]
        # early phase-2 tiles live outside the phase-1 pool footprint so
        # their first writes are not gated on phase-1 SBUF reuse
        b2b = wbulk.tile([P, D], FP32, tag="b2b", name="b2b")
        gb = wbulk.tile([P, D], FP32, tag="gb", name="gb")
        bb = wbulk.tile([P, D], FP32, tag="bb", name="bb")
        atp = top.enter_context(tc.tile_pool(name="atp", bufs=2))
        nrm = top.enter_context(tc.tile_pool(name="nrm", bufs=2))
        pptop = top.enter_context(tc.tile_pool(name="pptop", bufs=2, space="PSUM"))
        attnTbs = {}

        def normalize_emit(b):
            rec = nrm.tile([16, QT_TOK], FP32, tag="rec", name=f"rec_{b}")
            nc.vector.reciprocal(rec[:], denf[b][:])
            recb = nrm.tile([16, QT_TOK], BF, tag="recb", name=f"recb_{b}")
            nc.vector.tensor_copy(recb[:], rec[:])
            for i in range(8):
                rb = pptop.tile([P, 512], FP32, tag="mm", name=f"rb_{b}_{i}")
                nc.tensor.matmul(
                    rb[:, 0:QT_TOK],
                    lhsT=selm_sb[:, i, :],
                    rhs=recb[:],
                    start=True,
                    stop=True,
                )
                nc.vector.tensor_tensor(
                    attVf[b][:, i, :], attVf[b][:, i, :], rb[:, 0:QT_TOK], ALU.mult
                )
                nc.vector.tensor_scalar_add(
                    attVf[b][:, i, :], attVf[b][:, i, :], bv_sb[:, i : i + 1]
                )

        def wo_emit(b, m0, m1):
            if b not in attnTbs:
                attnTbs[b] = atp.tile([P, 8, QT_TOK], BF, tag="aT", name=f"aT_{b}")
            for m in range(m0, m1):
                ps = pptop.tile([P, 512], FP32, tag="mm", name=f"wo_{b}_{m}")
                for kc in range(8):
                    nc.tensor.matmul(
                        ps[:, 0:QT_TOK],
                        lhsT=wo_sb[:, kc, m * P : (m + 1) * P],
                        rhs=attVf[b][:, kc, :],
                        start=(kc == 0),
                        stop=(kc == 7),
                    )
                nc.vector.tensor_scalar_add(
                    attnTbs[b][:, m, :], ps[:, 0:QT_TOK], bo_sb[:, m : m + 1]
                )


        # =========================== phase 1 ===========================
        with ExitStack() as ph1:
            p1 = ph1.enter_context(tc.tile_pool(name="p1", bufs=1))
            QT = p1.tile([P, 2, S], BF, tag="QT")
            KT = p1.tile([P, 2, S], BF, tag="KT")
            Vp = p1.tile([P, 2, 16, 130], BF, tag="Vp")
            attV = p1.tile([P, 2, S], BF, tag="attV")
            attD = p1.tile([65, 2, S], BF, tag="attD")  # row 64 only, per batch

            for b in range(2):
                vv = Vp[:, b, :, :]
                nc.vector.memset(vv[:, :, 64:65], 1.0)
                nc.vector.memset(vv[:, :, 129:130], 1.0)

            xp = ph1.enter_context(tc.tile_pool(name="xp", bufs=3))
            pp_s = ph1.enter_context(tc.tile_pool(name="pps", bufs=2, space="PSUM"))
            pp_o = ph1.enter_context(tc.tile_pool(name="ppo", bufs=2, space="PSUM"))
            ptp = ph1.enter_context(tc.tile_pool(name="ptp", bufs=5))

            xts = {}

            xts[(0, 0)] = xt00

            def qkv_dma(b):
                # xT chunks for this batch (sync queue), token-group major
                for ng in range(4):
                    if (b, ng) in xts:
                        continue
                    xt = xp.tile([P, 8, 512], BF, tag="xt", name=f"xt_{b}_{ng}")
                    nc.sync.dma_start(xt[:], xT_d[:, b, :, ng * 512 : (ng + 1) * 512])
                    xts[(b, ng)] = xt

            def qk_ng(b, ng):
                # per token-group: Q, K -- each xT chunk is fully consumed
                # before the next is needed (xp ring stays small)
                if True:
                    for dst, w_sb, b_sb in ((QT, wq_sb, bq_sb), (KT, wk_sb, bk_sb)):
                        ps = pptop.tile([P, 512], FP32, tag="mm", name=f"qk_{b}_{ng}")
                        for kc in range(8):
                            nc.tensor.matmul(
                                ps[:, 0:512],
                                lhsT=w_sb[:, kc, :],
                                rhs=xts[(b, ng)][:, kc, :],
                                start=(kc == 0),
                                stop=(kc == 7),
                            )
                        nc.vector.tensor_scalar_add(
                            dst[:, b, ng * 512 : (ng + 1) * 512],
                            ps[:, 0:512],
                            b_sb[:, 0:1],
                        )
            def v_ng(b, ng):
                if True:
                    for tt in range(4 * ng, 4 * ng + 4):
                        ps = pptop.tile([P, 512], FP32, tag="mm", name=f"v_{b}_{tt}")
                        for kc in range(8):
                            nc.tensor.matmul(
                                ps[:, 0:P],
                                lhsT=xts[(b, ng)][
                                    :, kc, (tt % 4) * P : (tt % 4 + 1) * P
                                ],
                                rhs=wv_sb[:, kc, :],
                                start=(kc == 0),
                                stop=(kc == 7),
                            )
                        dst = Vp[:, b, tt, :].rearrange("p (h j) -> p h j", j=65)[
                            :, :, 0:64
                        ]
                        nc.vector.tensor_copy(
                            dst, ps[:, 0:P].rearrange("p (h j) -> p h j", j=64)
                        )

            def attn_g4(b, g4):
                if True:
                    o_pair = [
                        pp_o.tile([65, 512], FP32, tag="o", name=f"o_{b}_{g4}_{i}")
                        for i in range(2)
                    ]
                    nt = 4 * g4 + 4
                    for t in range(nt):
                        r = t - 4 * g4
                        qoff = max(r, 0) * P
                        s_pair = pp_s.tile(
                            [P, 1024], FP32, tag="s", name=f"s_{b}_{g4}_{t}"
                        )
                        for i, hp in enumerate((0, 64)):
                            nc.tensor.matmul(
                                s_pair[:, i * 512 + qoff : (i + 1) * 512],
                                lhsT=KT[hp : hp + 64, b, t * P : (t + 1) * P],
                                rhs=QT[
                                    hp : hp + 64,
                                    b,
                                    g4 * 512 + qoff : (g4 + 1) * 512,
                                ],
                                start=True,
                                stop=True,
                            )
                        if r >= 0:
                            # add -240 to masked entries of the diagonal block
                            # (exp -> ~1e-13) via PE, keeping DVE off the path
                            for i in range(2):
                                nc.tensor.matmul(
                                    s_pair[:, i * 512 + qoff : i * 512 + qoff + P],
                                    lhsT=eye_sb[:],
                                    rhs=mask_sb[:],
                                    start=False,
                                    stop=True,
                                    skip_group_check=True,
                                )
                        pt = ptp.tile([P, 1024], BF, tag="pt", name=f"pt_{b}_{g4}_{t}")
                        sv = s_pair[:].rearrange("p (h n) -> p h n", h=2)
                        pv = pt[:].rearrange("p (h n) -> p h n", h=2)
                        nc.scalar.activation(
                            pv[:, :, qoff:], sv[:, :, qoff:], AF.Exp, scale=0.125
                        )
                        for i in range(2):
                            nc.tensor.matmul(
                                o_pair[i][:, qoff:],
                                lhsT=Vp[:, b, t, i * 65 : (i + 1) * 65],
                                rhs=pt[:, i * 512 + qoff : (i + 1) * 512],
                                start=(t == 0),
                                stop=(t == nt - 1),
                            )
                    # unnormalized output + denominators (cast to bf16)
                    for i in range(2):
                        nc.vector.tensor_copy(
                            attV[i * 64 : (i + 1) * 64, b, g4 * 512 : (g4 + 1) * 512],
                            o_pair[i][0:64, :],
                        )
                        nc.vector.tensor_copy(
                            attD[64:65, i, g4 * 512 : (g4 + 1) * 512],
                            o_pair[i][64:65, :],
                        )

            def stage(b):
                vin = a2a_in[b][:].rearrange("(j r) q -> r j q", r=130)
                nc.sync.dma_start(
                    vin[0:P],
                    attV[:, b, :].rearrange("p (j q) -> p j q", q=QT_TOK),
                )
                for i in range(2):
                    nc.sync.dma_start(vin[P + i : P + i + 1], attD[64:65, i, :])
                if reps > 1:
                    # collectives cannot sit inside the timing repeat loop;
                    # substitute an equal-size local DMA (timing builds only)
                    nc.sync.dma_start(a2a_out[b][:], a2a_in[b][:])
                else:
                    nc.gpsimd.collective_compute(
                        "AllToAll",
                        ALU.bypass,
                        ins=[a2a_in[b][:]],
                        outs=[a2a_out[b][:]],
                        replica_groups=[[0, 1, 2, 3, 4, 5, 6, 7]],
                    )
                # receive on the SP queue (a trigger's sem wait blocks its
                # engine's whole stream -- SP is idle here, Act is not)
                vout = a2a_out[b][:].rearrange("(j r) q -> r j q", r=130)
                nc.sync.dma_start(attVf[b][:], vout[0:P])
                nc.sync.dma_start(
                    denf[b][:], vout[P : P + 2].rearrange("r j q -> j r q")
                )

            qkv_dma(0)
            qkv_dma(1)
            # bulk weights on the SP queue, behind the xT chunks
            nc.sync.dma_start(wo_sb[:], wo_d[:])
            for fg in range(8):
                nc.sync.dma_start(
                    w1_sb[:, :, fg * 512 : (fg + 1) * 512],
                    w1_d[:, :, fg * 512 : (fg + 1) * 512],
                )
            for ng in range(4):
                qk_ng(0, ng)
                v_ng(0, ng)
            # QKV b1 slots into attention b0's Act-bound PE gaps in small
            # chunks (big blocks starve the exp stream); the last chunk runs
            # during the A2A_0 staging so stage(0) fires as early as possible
            attn_g4(0, 0)
            qk_ng(1, 0)
            v_ng(1, 0)
            attn_g4(0, 1)
            qk_ng(1, 1)
            v_ng(1, 1)
            attn_g4(0, 2)
            qk_ng(1, 2)
            v_ng(1, 2)
            attn_g4(0, 3)
            stage(0)
            qk_ng(1, 3)
            v_ng(1, 3)
            for g4 in range(4):
                attn_g4(1, g4)
            stage(1)

        # =========================== phase 2 ===========================
        with ExitStack() as ph2:
            p2 = ph2.enter_context(tc.tile_pool(name="p2", bufs=1))
            w2p = ph2.enter_context(tc.tile_pool(name="w2p", bufs=2))
            lnp = ph2.enter_context(tc.tile_pool(name="lnp", bufs=2))
            smal = ph2.enter_context(tc.tile_pool(name="smal", bufs=2))
            pp_mm = ph2.enter_context(tc.tile_pool(name="ppmm2", bufs=2, space="PSUM"))
            pp_w2 = ph2.enter_context(tc.tile_pool(name="ppw2", bufs=2, space="PSUM"))

            hT = p2.tile([P, 32, QT_TOK], BF, tag="hT")
            attn_sb = p2.tile([P, 2, D], FP32, tag="attn")

            # broadcast rows b2/gamma/beta -> [128, 1024] fp32 (early, idle PE)
            for rp, dst in ((0, b2b), (32, gb), (64, bb)):
                for hf in range(2):
                    psb = pp_mm.tile([P, 512], FP32, tag="mm", name=f"bc_{rp}_{hf}")
                    nc.tensor.matmul(
                        psb[:],
                        lhsT=ones_sb[rp : rp + 1, :],
                        rhs=rows_sb[rp : rp + 1, hf * 512 : (hf + 1) * 512],
                        start=True,
                        stop=True,
                    )
                    nc.vector.tensor_copy(dst[:, hf * 512 : (hf + 1) * 512], psb[:])

            for b in range(2):
                normalize_emit(b)
                wo_emit(b, 0, 8)
                attnTb = attnTbs[b]

                # ---- W1 + exact GELU (+b1) -> hT[f, q] bf16 ----
                for f in range(32):
                    ps = pptop.tile([P, 512], FP32, tag="mm", name=f"w1_{b}_{f}")
                    for kc in range(8):
                        nc.tensor.matmul(
                            ps[:, 0:QT_TOK],
                            lhsT=w1_sb[:, kc, f * P : (f + 1) * P],
                            rhs=attnTb[:, kc, :],
                            start=(kc == 0),
                            stop=(kc == 7),
                        )
                    nc.scalar.activation(
                        hT[:, f, :],
                        ps[:, 0:QT_TOK],
                        AF.Gelu,
                        bias=b1_sb[:, f : f + 1],
                        scale=1.0,
                    )

                # ---- transpose attnTb -> attn_sb[q, dout] fp32 (+b2) ----
                for m in range(8):
                    for t2 in range(2):
                        pst = pp_mm.tile([P, QT_TOK], BF, tag="tr", name=f"tr_{b}_{m}_{t2}")
                        nc.tensor.transpose(
                            pst[:, 0:P],
                            attnTb[:, m, t2 * P : (t2 + 1) * P],
                            eye_sb[:],
                        )
                        nc.vector.tensor_tensor(
                            attn_sb[:, t2, m * P : (m + 1) * P],
                            pst[:, 0:P],
                            b2b[:, m * P : (m + 1) * P],
                            ALU.add,
                        )

                # ---- W2 + residual -> attn_sb (in place), LN pipelined:
                # row-sum/sum-of-squares computed per 512-col half as soon as
                # that half's residual add lands ----
                def ln_partial(b, t2, ng):
                    yh = attn_sb[:, t2, ng * 512 : (ng + 1) * 512]
                    s1p = smal.tile([P, 2], FP32, tag=f"s1p{ng}", name=f"s1p_{b}_{t2}_{ng}")
                    nc.vector.reduce_sum(s1p[:, 0:1], yh, axis=AX.X)
                    sqo = lnp.tile([P, D], FP32, tag="sc", name=f"sq_{b}_{t2}_{ng}")
                    nc.scalar.activation(
                        sqo[:, 0:512], yh, AF.Square, accum_out=s1p[:, 1:2]
                    )
                    return s1p

                def ln_out(b, t2, parts):
                    yv = attn_sb[:, t2, :]
                    s1 = smal.tile([P, 1], FP32, tag="s1", name=f"s1_{b}_{t2}")
                    nc.vector.tensor_tensor(
                        s1[:], parts[0][:, 0:1], parts[1][:, 0:1], ALU.add
                    )
                    s2 = smal.tile([P, 1], FP32, tag="s2", name=f"s2_{b}_{t2}")
                    nc.vector.tensor_tensor(
                        s2[:], parts[0][:, 1:2], parts[1][:, 1:2], ALU.add
                    )
                    negmean = smal.tile([P, 1], FP32, tag="nm", name=f"nm_{b}_{t2}")
                    nc.vector.tensor_scalar_mul(negmean[:], s1[:], -1.0 / D)
                    mm2 = smal.tile([P, 1], FP32, tag="mm2", name=f"m2_{b}_{t2}")
                    nc.vector.tensor_tensor(mm2[:], negmean[:], negmean[:], ALU.mult)
                    bap = smal.tile([P, 1], FP32, tag="bap", name=f"ba_{b}_{t2}")
                    nc.vector.tensor_scalar(
                        bap[:], mm2[:], -1.0, 1e-6, ALU.mult, ALU.add
                    )
                    std = smal.tile([P, 1], FP32, tag="std", name=f"sd_{b}_{t2}")
                    nc.scalar.activation(
                        std[:], s2[:], AF.Sqrt, bias=bap[:], scale=1.0 / D
                    )
                    rstd = smal.tile([P, 1], FP32, tag="rstd", name=f"rs_{b}_{t2}")
                    nc.vector.reciprocal(rstd[:], std[:])
                    t1 = lnp.tile([P, D], FP32, tag="sc", name=f"t1_{b}_{t2}")
                    nc.vector.tensor_scalar(
                        t1[:], yv, negmean[:], rstd[:], ALU.add, ALU.mult
                    )
                    nc.vector.tensor_tensor(t1[:], t1[:], gb[:], ALU.mult)
                    nc.vector.tensor_tensor(t1[:], t1[:], bb[:], ALU.add)
                    nc.sync.dma_start(
                        out_d[(2 * b + t2) * P : (2 * b + t2 + 1) * P, :], t1[:]
                    )

                parts = {}
                for ng in range(2):
                    w2_half = []
                    for hf in range(2):
                        w2t = w2p.tile(
                            [P, 16, 512], BF, tag="w2", name=f"w2_{b}_{ng}_{hf}"
                        )
                        nc.scalar.dma_start(
                            w2t[:],
                            w2_d[
                                :,
                                hf * 16 : (hf + 1) * 16,
                                ng * 512 : (ng + 1) * 512,
                            ],
                        )
                        w2_half.append(w2t)
                    for t2 in range(2):
                        ps = pp_w2.tile([P, 512], FP32, tag="w2", name=f"w2p_{b}_{ng}_{t2}")
                        for fc in range(32):
                            nc.tensor.matmul(
                                ps[:],
                                lhsT=hT[:, fc, t2 * P : (t2 + 1) * P],
                                rhs=w2_half[fc // 16][:, fc % 16, :],
                                start=(fc == 0),
                                stop=(fc == 31),
                            )
                        nc.vector.tensor_tensor(
                            attn_sb[:, t2, ng * 512 : (ng + 1) * 512],
                            ps[:],
                            attn_sb[:, t2, ng * 512 : (ng + 1) * 512],
                            ALU.add,
                        )
                        parts[(t2, ng)] = ln_partial(b, t2, ng)
                        if ng == 1:
                            ln_out(b, t2, (parts[(t2, 0)], parts[(t2, 1)]))

    _split_multi_waits(nc)
    return nc


_CACHE = {}


def _get_nc(reps=1):
    key = ("nc", reps)
    if key not in _CACHE:
        _CACHE[key] = _build_nc(reps)
    return _CACHE[key]


def _prep_in_maps(x, mask, Wq, bq, Wk, bk, Wv, bv, Wo, bo, W1, b1, W2, b2, gamma, beta):
    x = np.asarray(x, np.float32)

    def chunkT(w, nch):
        return np.ascontiguousarray(
            np.asarray(w, np.float32).astype(BF16).reshape(nch, P, -1).transpose(1, 0, 2)
        )

    wo_h = chunkT(Wo, 8)
    w1_h = chunkT(W1, 8)
    w2_h = chunkT(W2, 32)
    Wq = np.asarray(Wq, np.float32)
    Wk = np.asarray(Wk, np.float32)
    Wv = np.asarray(Wv, np.float32)

    def bT(b_, nch):
        return np.ascontiguousarray(np.asarray(b_, np.float32).reshape(nch, P).T)

    bo_h = bT(bo, 8)
    b1_h = bT(b1, 32)
    bv_h = bT(bv, 8)
    rows_h = np.zeros((65, D), np.float32)
    rows_h[0] = np.asarray(b2, np.float32)
    rows_h[32] = np.asarray(gamma, np.float32)
    rows_h[64] = np.asarray(beta, np.float32)
    cb_h = np.concatenate(
        [_tri_mask(), np.eye(P, dtype=np.float32).astype(BF16)], axis=1
    )
    bq = np.asarray(bq, np.float32)
    bk = np.asarray(bk, np.float32)

    # xT[p, b, kc, s] = x[b, s, kc*128+p]
    xT_h = np.ascontiguousarray(
        x.transpose(2, 0, 1).astype(BF16).reshape(8, P, 2, S).transpose(1, 2, 0, 3)
    )
    # selm[r, i, p] = 1 if r == 2i + (p>=64)
    selm_h = np.zeros((16, 8, P), np.float32)
    for i in range(8):
        selm_h[2 * i, i, 0:64] = 1.0
        selm_h[2 * i + 1, i, 64:P] = 1.0
    selm_h = selm_h.astype(BF16)

    in_maps = []
    for k in range(NCORES):
        hs = slice(k * P, (k + 1) * P)
        cf_h = np.zeros((P, 50), np.float32)
        cf_h[:, 0] = bq[hs]
        cf_h[:, 1] = bk[hs]
        cf_h[:, 2:10] = bv_h
        cf_h[:, 10:18] = bo_h
        cf_h[:, 18:50] = b1_h
        in_maps.append(
            {
                "xT": xT_h,
                "cb": cb_h,
                "cf": cf_h,
                "selm": selm_h,
                "wq": chunkT(Wq[:, hs], 8),
                "wk": chunkT(Wk[:, hs], 8),
                "wv": chunkT(Wv[:, hs], 8),
                "wo": wo_h,
                "w1": w1_h,
                "w2": w2_h,
                "rows": rows_h,
            }
        )
    return in_maps


def kernel(**inputs):
    in_maps = _prep_in_maps(**inputs)
    nc = _get_nc()
    res = run_bass_kernel_spmd(nc, in_maps, core_ids=list(range(NCORES)))
    out = np.zeros((B, S, D), np.float32)
    for k in range(NCORES):
        o = res.results[k]["out"]  # [512, D]: batch0 tokens then batch1
        out[0, k * 2 * P : (k + 1) * 2 * P] = o[0 : 2 * P]
        out[1, k * 2 * P : (k + 1) * 2 * P] = o[2 * P : 4 * P]
    return out


# revision 55
# speedup vs baseline: 1.0081x; 1.0081x over previous
"""Trainium2 Bass kernel for nn_DecoderLayer_68212670595779.

Sharding: each of the 8 cores owns one head PAIR (heads 2k, 2k+1) for BOTH
batches, and (batch, token-slice k*256:(k+1)*256) for the FFN. Attention
output ships UNNORMALIZED through a per-batch 8-rank AllToAll with the
softmax denominators as 2 extra payload rows per shard; normalization
happens post-collective. The batch-0 collective hides under batch-1
attention; the batch-1 collective hides under the batch-0 FFN half.
"""
import sys

sys.path.insert(0, "/opt/trn_rl_repo")

import numpy as np
import ml_dtypes
from contextlib import ExitStack

import concourse.bass as bass
import concourse.mybir as mybir
import concourse.tile as tile
from concourse.vector_clock import ScopedClock
from concourse.bass_utils import run_bass_kernel_spmd

BF16 = ml_dtypes.bfloat16
FP32 = mybir.dt.float32
BF = mybir.dt.bfloat16
AF = mybir.ActivationFunctionType
ALU = mybir.AluOpType
AX = mybir.AxisListType

B, S, D, H, HD, FF, P = 2, 2048, 1024, 16, 64, 4096, 128
NCORES = 8
QT_TOK = 256  # tokens per (core, batch) in phase 2


# ---------------------------------------------------------------------------
# Workaround: this walrus build allows only ONE semaphore wait on a CTRL
# (Drain) instruction; TileContext's final drain carries one wait per busy
# proc. Split the waits across a chain of drains on the same engine.
def _patched_drain_and_barrier(self, tick_clock, wait_clock):
    nc = self.nc
    drain_inst = nc.sync.drain()
    wait_clock.add_sem_waits(
        drain_inst.ins, ScopedClock({None: tick_clock.global_clock})
    )
    si = drain_inst.ins.sync_info
    waits = list(si.on_wait) if si is not None else []
    if len(waits) > 1:
        si.on_wait = waits[:1]
        for w in waits[1:]:
            extra = nc.sync.drain()
            esi = extra.ins.sync_info
            if esi is None:
                extra.ins.sync_info = mybir.SyncInfo(on_wait=[w], on_update=[])
            else:
                esi.on_wait = [w]
    nc.all_engine_barrier()
    assert self.sems is not None
    popped = nc._tile_sem_poison_stack.pop()
    assert popped is self._sem_poison
    nc.clear_and_free_semaphores(list(self.sems.allocated().values()))
    nc.all_engine_barrier()


tile.TileContext._drain_and_barrier = _patched_drain_and_barrier


def _split_multi_waits(nc):
    """Walrus in this container supports a single sem wait per instruction.
    Move extra waits onto dedicated no-op instructions on the same engine,
    inserted immediately before (engine program order preserves semantics)."""
    n_split = 0
    for fn in nc.m.functions:
        for bb in fn.blocks:
            out = []
            for ins in bb.instructions:
                si = ins.sync_info
                waits = list(si.on_wait) if si is not None else []
                if len(waits) > 1:
                    si.on_wait = [waits[-1]]
                    for i, w in enumerate(waits[:-1]):
                        nop = mybir.InstNoOp(
                            name=f"{ins.name}-sw{i}",
                            engine=ins.engine,
                            bass_nofuse=True,
                            sync_info=mybir.SyncInfo(on_wait=[w], on_update=[]),
                        )
                        out.append(nop)
                        n_split += 1
                out.append(ins)
            bb.instructions[:] = out
    return n_split


def _tri_mask():
    """[kv 128, q 128]: -240 where kv > q (masked), else 0. Added to scores
    pre-exp: exp(0.125*(s-240)) ~ 1e-13, i.e. effectively zero weight."""
    a = np.arange(P)[:, None]
    qq = np.arange(P)[None, :]
    return np.where(a > qq, -240.0, 0.0).astype(np.float32).astype(BF16)


def _build_nc(reps=1):
    nc = bass.Bass()

    def din(name, shape, dt=BF):
        return nc.declare_dram_parameter(name, list(shape), dt, isOutput=False)

    xT_d = din("xT", (P, 2, 8, S))          # x[b].T, bf16, kc-chunked
    cb_d = din("cb", (P, 2 * P))            # packed bf16: mask | eye
    cf_d = din("cf", (P, 50), FP32)         # packed fp32: bq|bk|bv|bo|b1
    wq_d = din("wq", (P, 8, P))             # Wq[:, 2k*64:(2k+2)*64] kc-chunked
    wk_d = din("wk", (P, 8, P))
    wv_d = din("wv", (P, 8, P))
    wo_d = din("wo", (P, 8, D))
    w1_d = din("w1", (P, 8, FF))
    w2_d = din("w2", (P, 32, D))
    selm_d = din("selm", (16, 8, P))        # denom-row selection matrices
    rows_d = din("rows", (65, D), FP32)     # b2@0 / gamma@32 / beta@64
    out_d = nc.declare_dram_parameter("out", [2 * QT_TOK * 2, D], FP32, isOutput=True)
    # per-batch A2A buffers: 8 shards x (128 dim rows + 2 denom rows) x 256 tok
    a2a_in = [nc.dram_tensor(f"a2ain{b}", [8 * 130, QT_TOK], BF) for b in range(2)]
    a2a_out = [nc.dram_tensor(f"a2aout{b}", [8 * 130, QT_TOK], BF) for b in range(2)]

    with ExitStack() as top:
        tc = top.enter_context(tile.TileContext(nc))

        const = top.enter_context(tc.tile_pool(name="const", bufs=1))

        # ---- first xT chunk ahead of everything (feeds the first matmul);
        # constants packed into few DMAs (dispatch overhead matters) ----
        xp0 = top.enter_context(tc.tile_pool(name="xp0", bufs=1))
        xt00 = xp0.tile([P, 8, 512], BF, tag="xt00")
        nc.sync.dma_start(xt00[:, 0:2, :], xT_d[:, 0, 0:2, 0:512])
        nc.sync.dma_start(xt00[:, 2:8, :], xT_d[:, 0, 2:8, 0:512])
        cb_sb = const.tile([P, 2 * P], BF, tag="cb")
        nc.sync.dma_start(cb_sb[:], cb_d[:])
        cf_sb = const.tile([P, 50], FP32, tag="cf")
        nc.sync.dma_start(cf_sb[:], cf_d[:])
        ones_sb = const.tile([P, P], FP32, tag="ones")
        nc.vector.memset(ones_sb[:], 1.0)
        mask_sb = cb_sb[:, 0:P]
        eye_sb = cb_sb[:, P : 2 * P]
        bq_sb = cf_sb[:, 0:1]
        bk_sb = cf_sb[:, 1:2]
        bv_sb = cf_sb[:, 2:10]
        bo_sb = cf_sb[:, 10:18]
        b1_sb = cf_sb[:, 18:50]

        if reps > 1:
            # timing-only variant: repeat the whole body on-device so HW time
            # dominates host/tunnel dispatch overhead
            top.enter_context(tc.For_i(0, reps, 1))

        # ---- QKV weights (Act HWDGE queue, needed immediately) ----
        wbulk = top.enter_context(tc.tile_pool(name="wbulk", bufs=1))
        wq_sb = wbulk.tile([P, 8, P], BF, tag="wq")
        nc.scalar.dma_start(wq_sb[:, 0:2, :], wq_d[:, 0:2, :])
        nc.scalar.dma_start(wq_sb[:, 2:8, :], wq_d[:, 2:8, :])
        wk_sb = wbulk.tile([P, 8, P], BF, tag="wk")
        nc.scalar.dma_start(wk_sb[:], wk_d[:])
        wv_sb = wbulk.tile([P, 8, P], BF, tag="wv")
        nc.scalar.dma_start(wv_sb[:], wv_d[:])
        selm_sb = wbulk.tile([16, 8, P], BF, tag="selm", name="selm_sb")
        nc.scalar.dma_start(selm_sb[:], selm_d[:])
        rows_sb = wbulk.tile([65, D], FP32, tag="rows", name="rows_sb")
        nc.scalar.dma_start(rows_sb[:], rows_d[:])
        wo_sb = wbulk.tile([P, 8, D], BF, tag="wo")
        w1_sb = wbulk.tile([P, 8, FF], BF, tag="w1")
        # cross-phase tiles for collective receive (emitted in phase 1)
        attVf = [
            wbulk.tile([P, 8, QT_TOK], BF, tag=f"vf{b}", name=f"attVf{b}")
            for b in range(2)
        ]
        denf = [
            wbulk.tile([16, QT_TOK], BF, tag=f"den{b}", name=f"denf{b}")
            for b in range(2)
        ]
        # early phase-2 tiles live outside the phase-1 pool footprint so
        # their first writes are not gated on phase-1 SBUF reuse
        b2b = wbulk.tile([P, D], FP32, tag="b2b", name="b2b")
        gb = wbulk.tile([P, D], FP32, tag="gb", name="gb")
        bb = wbulk.tile([P, D], FP32, tag="bb", name="bb")
        atp = top.enter_context(tc.tile_pool(name="atp", bufs=2))
        nrm = top.enter_context(tc.tile_pool(name="nrm", bufs=2))
        pptop = top.enter_context(tc.tile_pool(name="pptop", bufs=2, space="PSUM"))
        attnTbs = {}

        def normalize_emit(b):
            rec = nrm.tile([16, QT_TOK], FP32, tag="rec", name=f"rec_{b}")
            nc.vector.reciprocal(rec[:], denf[b][:])
            recb = nrm.tile([16, QT_TOK], BF, tag="recb", name=f"recb_{b}")
            nc.vector.tensor_copy(recb[:], rec[:])
            for i in range(8):
                rb = pptop.tile([P, 512], FP32, tag="mm", name=f"rb_{b}_{i}")
                nc.tensor.matmul(
                    rb[:, 0:QT_TOK],
                    lhsT=selm_sb[:, i, :],
                    rhs=recb[:],
                    start=True,
                    stop=True,
                )
                nc.vector.tensor_tensor(
                    attVf[b][:, i, :], attVf[b][:, i, :], rb[:, 0:QT_TOK], ALU.mult
                )
                nc.vector.tensor_scalar_add(
                    attVf[b][:, i, :], attVf[b][:, i, :], bv_sb[:, i : i + 1]
                )

        def wo_emit(b, m0, m1):
            if b not in attnTbs:
                attnTbs[b] = atp.tile([P, 8, QT_TOK], BF, tag="aT", name=f"aT_{b}")
            for m in range(m0, m1):
                ps = pptop.tile([P, 512], FP32, tag="mm", name=f"wo_{b}_{m}")
                for kc in range(8):
                    nc.tensor.matmul(
                        ps[:, 0:QT_TOK],
                        lhsT=wo_sb[:, kc, m * P : (m + 1) * P],
                        rhs=attVf[b][:, kc, :],
                        start=(kc == 0),
                        stop=(kc == 7),
                    )
                nc.vector.tensor_scalar_add(
                    attnTbs[b][:, m, :], ps[:, 0:QT_TOK], bo_sb[:, m : m + 1]
                )


        # =========================== phase 1 ===========================
        with ExitStack() as ph1:
            p1 = ph1.enter_context(tc.tile_pool(name="p1", bufs=1))
            QT = p1.tile([P, 2, S], BF, tag="QT")
            KT = p1.tile([P, 2, S], BF, tag="KT")
            Vp = p1.tile([P, 2, 16, 130], BF, tag="Vp")
            attV = p1.tile([P, 2, S], BF, tag="attV")
            attD = p1.tile([65, 2, S], BF, tag="attD")  # row 64 only, per batch

            for b in range(2):
                vv = Vp[:, b, :, :]
                nc.vector.memset(vv[:, :, 64:65], 1.0)
                nc.vector.memset(vv[:, :, 129:130], 1.0)

            xp = ph1.enter_context(tc.tile_pool(name="xp", bufs=3))
            pp_s = ph1.enter_context(tc.tile_pool(name="pps", bufs=2, space="PSUM"))
            pp_o = ph1.enter_context(tc.tile_pool(name="ppo", bufs=2, space="PSUM"))
            ptp = ph1.enter_context(tc.tile_pool(name="ptp", bufs=5))

            xts = {}

            xts[(0, 0)] = xt00

            def qkv_dma(b):
                # xT chunks for this batch (sync queue), token-group major
                for ng in range(4):
                    if (b, ng) in xts:
                        continue
                    xt = xp.tile([P, 8, 512], BF, tag="xt", name=f"xt_{b}_{ng}")
                    nc.sync.dma_start(xt[:], xT_d[:, b, :, ng * 512 : (ng + 1) * 512])
                    xts[(b, ng)] = xt

            def qk_ng(b, ng):
                # per token-group: Q, K -- each xT chunk is fully consumed
                # before the next is needed (xp ring stays small)
                if True:
                    for dst, w_sb, b_sb in ((QT, wq_sb, bq_sb), (KT, wk_sb, bk_sb)):
                        ps = pptop.tile([P, 512], FP32, tag="mm", name=f"qk_{b}_{ng}")
                        for kc in range(8):
                            nc.tensor.matmul(
                                ps[:, 0:512],
                                lhsT=w_sb[:, kc, :],
                                rhs=xts[(b, ng)][:, kc, :],
                                start=(kc == 0),
                                stop=(kc == 7),
                            )
                        nc.vector.tensor_scalar_add(
                            dst[:, b, ng * 512 : (ng + 1) * 512],
                            ps[:, 0:512],
                            b_sb[:, 0:1],
                        )
            def v_ng(b, ng):
                if True:
                    for tt in range(4 * ng, 4 * ng + 4):
                        ps = pptop.tile([P, 512], FP32, tag="mm", name=f"v_{b}_{tt}")
                        for kc in range(8):
                            nc.tensor.matmul(
                                ps[:, 0:P],
                                lhsT=xts[(b, ng)][
                                    :, kc, (tt % 4) * P : (tt % 4 + 1) * P
                                ],
                                rhs=wv_sb[:, kc, :],
                                start=(kc == 0),
                                stop=(kc == 7),
                            )
                        dst = Vp[:, b, tt, :].rearrange("p (h j) -> p h j", j=65)[
                            :, :, 0:64
                        ]
                        nc.vector.tensor_copy(
                            dst, ps[:, 0:P].rearrange("p (h j) -> p h j", j=64)
                        )

            def attn_g4(b, g4):
                if True:
                    o_pair = [
                        pp_o.tile([65, 512], FP32, tag="o", name=f"o_{b}_{g4}_{i}")
                        for i in range(2)
                    ]
                    nt = 4 * g4 + 4
                    for t in range(nt):
                        r = t - 4 * g4
                        qoff = max(r, 0) * P
                        s_pair = pp_s.tile(
                            [P, 1024], FP32, tag="s", name=f"s_{b}_{g4}_{t}"
                        )
                        for i, hp in enumerate((0, 64)):
                            nc.tensor.matmul(
                                s_pair[:, i * 512 + qoff : (i + 1) * 512],
                                lhsT=KT[hp : hp + 64, b, t * P : (t + 1) * P],
                                rhs=QT[
                                    hp : hp + 64,
                                    b,
                                    g4 * 512 + qoff : (g4 + 1) * 512,
                                ],
                                start=True,
                                stop=True,
                            )
                        if r >= 0:
                            # add -240 to masked entries of the diagonal block
                            # (exp -> ~1e-13) via PE, keeping DVE off the path
                            for i in range(2):
                                nc.tensor.matmul(
                                    s_pair[:, i * 512 + qoff : i * 512 + qoff + P],
                                    lhsT=eye_sb[:],
                                    rhs=mask_sb[:],
                                    start=False,
                                    stop=True,
                                    skip_group_check=True,
                                )
                        pt = ptp.tile([P, 1024], BF, tag="pt", name=f"pt_{b}_{g4}_{t}")
                        sv = s_pair[:].rearrange("p (h n) -> p h n", h=2)
                        pv = pt[:].rearrange("p (h n) -> p h n", h=2)
                        nc.scalar.activation(
                            pv[:, :, qoff:], sv[:, :, qoff:], AF.Exp, scale=0.125
                        )
                        for i in range(2):
                            nc.tensor.matmul(
                                o_pair[i][:, qoff:],
                                lhsT=Vp[:, b, t, i * 65 : (i + 1) * 65],
                                rhs=pt[:, i * 512 + qoff : (i + 1) * 512],
                                start=(t == 0),
                                stop=(t == nt - 1),
                            )
                    # unnormalized output + denominators (cast to bf16)
                    for i in range(2):
                        nc.vector.tensor_copy(
                            attV[i * 64 : (i + 1) * 64, b, g4 * 512 : (g4 + 1) * 512],
                            o_pair[i][0:64, :],
                        )
                        nc.vector.tensor_copy(
                            attD[64:65, i, g4 * 512 : (g4 + 1) * 512],
                            o_pair[i][64:65, :],
                        )

            def stage(b):
                vin = a2a_in[b][:].rearrange("(j r) q -> r j q", r=130)
                nc.sync.dma_start(
                    vin[0:P],
                    attV[:, b, :].rearrange("p (j q) -> p j q", q=QT_TOK),
                )
                for i in range(2):
                    nc.sync.dma_start(vin[P + i : P + i + 1], attD[64:65, i, :])
                if reps > 1:
                    # collectives cannot sit inside the timing repeat loop;
                    # substitute an equal-size local DMA (timing builds only)
                    nc.sync.dma_start(a2a_out[b][:], a2a_in[b][:])
                else:
                    nc.gpsimd.collective_compute(
                        "AllToAll",
                        ALU.bypass,
                        ins=[a2a_in[b][:]],
                        outs=[a2a_out[b][:]],
                        replica_groups=[[0, 1, 2, 3, 4, 5, 6, 7]],
                    )
                # receive on the SP queue (a trigger's sem wait blocks its
                # engine's whole stream -- SP is idle here, Act is not)
                vout = a2a_out[b][:].rearrange("(j r) q -> r j q", r=130)
                nc.sync.dma_start(attVf[b][:], vout[0:P])
                nc.sync.dma_start(
                    denf[b][:], vout[P : P + 2].rearrange("r j q -> j r q")
                )

            qkv_dma(0)
            qkv_dma(1)
            # bulk weights on the SP queue, behind the xT chunks
            nc.sync.dma_start(wo_sb[:], wo_d[:])
            for fg in range(8):
                nc.sync.dma_start(
                    w1_sb[:, :, fg * 512 : (fg + 1) * 512],
                    w1_d[:, :, fg * 512 : (fg + 1) * 512],
                )
            for ng in range(4):
                qk_ng(0, ng)
                v_ng(0, ng)
            # QKV b1 slots into attention b0's Act-bound PE gaps in small
            # chunks (big blocks starve the exp stream); the last chunk runs
            # during the A2A_0 staging so stage(0) fires as early as possible
            attn_g4(0, 0)
            qk_ng(1, 0)
            v_ng(1, 0)
            attn_g4(0, 1)
            qk_ng(1, 1)
            v_ng(1, 1)
            attn_g4(0, 2)
            qk_ng(1, 2)
            v_ng(1, 2)
            attn_g4(0, 3)
            stage(0)
            qk_ng(1, 3)
            v_ng(1, 3)
            for g4 in range(4):
                attn_g4(1, g4)
            stage(1)

        # =========================== phase 2 ===========================
        with ExitStack() as ph2:
            p2 = ph2.enter_context(tc.tile_pool(name="p2", bufs=1))
            w2p = ph2.enter_context(tc.tile_pool(name="w2p", bufs=2))
            lnp = ph2.enter_context(tc.tile_pool(name="lnp", bufs=2))
            smal = ph2.enter_context(tc.tile_pool(name="smal", bufs=2))
            pp_mm = ph2.enter_context(tc.tile_pool(name="ppmm2", bufs=2, space="PSUM"))
            pp_w2 = ph2.enter_context(tc.tile_pool(name="ppw2", bufs=2, space="PSUM"))

            hT = p2.tile([P, 32, QT_TOK], BF, tag="hT")
            attn_sb = p2.tile([P, 2, D], FP32, tag="attn")

            # broadcast rows b2/gamma/beta -> [128, 1024] fp32 (early, idle PE)
            for rp, dst in ((0, b2b), (32, gb), (64, bb)):
                for hf in range(2):
                    psb = pp_mm.tile([P, 512], FP32, tag="mm", name=f"bc_{rp}_{hf}")
                    nc.tensor.matmul(
                        psb[:],
                        lhsT=ones_sb[rp : rp + 1, :],
                        rhs=rows_sb[rp : rp + 1, hf * 512 : (hf + 1) * 512],
                        start=True,
                        stop=True,
                    )
                    nc.vector.tensor_copy(dst[:, hf * 512 : (hf + 1) * 512], psb[:])

            for b in range(2):
                normalize_emit(b)
                wo_emit(b, 0, 8)
                attnTb = attnTbs[b]

                # ---- W1 + exact GELU (+b1) -> hT[f, q] bf16 ----
                for f in range(32):
                    ps = pptop.tile([P, 512], FP32, tag="mm", name=f"w1_{b}_{f}")
                    for kc in range(8):
                        nc.tensor.matmul(
                            ps[:, 0:QT_TOK],
                            lhsT=w1_sb[:, kc, f * P : (f + 1) * P],
                            rhs=attnTb[:, kc, :],
                            start=(kc == 0),
                            stop=(kc == 7),
                        )
                    nc.scalar.activation(
                        hT[:, f, :],
                        ps[:, 0:QT_TOK],
                        AF.Gelu,
                        bias=b1_sb[:, f : f + 1],
                        scale=1.0,
                    )

                # ---- transpose attnTb -> attn_sb[q, dout] fp32 (+b2) ----
                for m in range(8):
                    for t2 in range(2):
                        pst = pp_mm.tile([P, QT_TOK], BF, tag="tr", name=f"tr_{b}_{m}_{t2}")
                        nc.tensor.transpose(
                            pst[:, 0:P],
                            attnTb[:, m, t2 * P : (t2 + 1) * P],
                            eye_sb[:],
                        )
                        nc.vector.tensor_tensor(
                            attn_sb[:, t2, m * P : (m + 1) * P],
                            pst[:, 0:P],
                            b2b[:, m * P : (m + 1) * P],
                            ALU.add,
                        )

                # ---- W2 + residual -> attn_sb (in place), LN pipelined:
                # row-sum/sum-of-squares computed per 512-col half as soon as
                # that half's residual add lands ----
                def ln_partial(b, t2, ng):
                    yh = attn_sb[:, t2, ng * 512 : (ng + 1) * 512]
                    s1p = smal.tile([P, 2], FP32, tag=f"s1p{ng}", name=f"s1p_{b}_{t2}_{ng}")
                    nc.vector.reduce_sum(s1p[:, 0:1], yh, axis=AX.X)
                    sqo = lnp.tile([P, D], FP32, tag="sc", name=f"sq_{b}_{t2}_{ng}")
                    nc.scalar.activation(
                        sqo[:, 0:512], yh, AF.Square, accum_out=s1p[:, 1:2]
                    )
                    return s1p

                def ln_out(b, t2, parts):
                    yv = attn_sb[:, t2, :]
                    s1 = smal.tile([P, 1], FP32, tag="s1", name=f"s1_{b}_{t2}")
                    nc.vector.tensor_tensor(
                        s1[:], parts[0][:, 0:1], parts[1][:, 0:1], ALU.add
                    )
                    s2 = smal.tile([P, 1], FP32, tag="s2", name=f"s2_{b}_{t2}")
                    nc.vector.tensor_tensor(
                        s2[:], parts[0][:, 1:2], parts[1][:, 1:2], ALU.add
                    )
                    negmean = smal.tile([P, 1], FP32, tag="nm", name=f"nm_{b}_{t2}")
                    nc.vector.tensor_scalar_mul(negmean[:], s1[:], -1.0 / D)
                    mm2 = smal.tile([P, 1], FP32, tag="mm2", name=f"m2_{b}_{t2}")
                    nc.vector.tensor_tensor(mm2[:], negmean[:], negmean[:], ALU.mult)
                    bap = smal.tile([P, 1], FP32, tag="bap", name=f"ba_{b}_{t2}")
                    nc.vector.tensor_scalar(
                        bap[:], mm2[:], -1.0, 1e-6, ALU.mult, ALU.add
                    )
                    std = smal.tile([P, 1], FP32, tag="std", name=f"sd_{b}_{t2}")
                    nc.scalar.activation(
                        std[:], s2[:], AF.Sqrt, bias=bap[:], scale=1.0 / D
                    )
                    rstd = smal.tile([P, 1], FP32, tag="rstd", name=f"rs_{b}_{t2}")
                    nc.vector.reciprocal(rstd[:], std[:])
                    t1 = lnp.tile([P, D], FP32, tag="sc", name=f"t1_{b}_{t2}")
                    nc.vector.tensor_scalar(
                        t1[:], yv, negmean[:], rstd[:], ALU.add, ALU.mult
                    )
                    nc.vector.tensor_tensor(t1[:], t1[:], gb[:], ALU.mult)
                    nc.vector.tensor_tensor(t1[:], t1[:], bb[:], ALU.add)
                    nc.sync.dma_start(
                        out_d[(2 * b + t2) * P : (2 * b + t2 + 1) * P, :], t1[:]
                    )

                parts = {}
                for ng in range(2):
                    w2_half = []
                    for hf in range(2):
                        w2t = w2p.tile(
                            [P, 16, 512], BF, tag="w2", name=f"w2_{b}_{ng}_{hf}"
                        )
                        nc.scalar.dma_start(
                            w2t[:],
                            w2_d[
                                :,
                                hf * 16 : (hf + 1) * 16,
                                ng * 512 : (ng + 1) * 512,
                            ],
                        )
                        w2_half.append(w2t)
                    for t2 in range(2):
                        ps = pp_w2.tile([P, 512], FP32, tag="w2", name=f"w2p_{b}_{ng}_{t2}")
                        for fc in range(32):
                            nc.tensor.matmul(
                                ps[:],
                                lhsT=hT[:, fc, t2 * P : (t2 + 1) * P],
                                rhs=w2_half[fc // 16][:, fc % 16, :],
                                start=(fc == 0),
                                stop=(fc == 31),
                            )
                        nc.vector.tensor_tensor(
                            attn_sb[:, t2, ng * 512 : (ng + 1) * 512],
                            ps[:],
                            attn_sb[:, t2, ng * 512 : (ng + 1) * 512],
                            ALU.add,
                        )
                        parts[(t2, ng)] = ln_partial(b, t2, ng)
                        if ng == 1:
                            ln_out(b, t2, (parts[(t2, 0)], parts[(t2, 1)]))

    _split_multi_waits(nc)
    return nc


_CACHE = {}


def _get_nc(reps=1):
    key = ("nc", reps)
    if key not in _CACHE:
        _CACHE[key] = _build_nc(reps)
    return _CACHE[key]


def _prep_in_maps(x, mask, Wq, bq, Wk, bk, Wv, bv, Wo, bo, W1, b1, W2, b2, gamma, beta):
    x = np.asarray(x, np.float32)

    def chunkT(w, nch):
        return np.ascontiguousarray(
            np.asarray(w, np.float32).astype(BF16).reshape(nch, P, -1).transpose(1, 0, 2)
        )

    wo_h = chunkT(Wo, 8)
    w1_h = chunkT(W1, 8)
    w2_h = chunkT(W2, 32)
    Wq = np.asarray(Wq, np.float32)
    Wk = np.asarray(Wk, np.float32)
    Wv = np.asarray(Wv, np.float32)

    def bT(b_, nch):
        return np.ascontiguousarray(np.asarray(b_, np.float32).reshape(nch, P).T)

    bo_h = bT(bo, 8)
    b1_h = bT(b1, 32)
    bv_h = bT(bv, 8)
    rows_h = np.zeros((65, D), np.float32)
    rows_h[0] = np.asarray(b2, np.float32)
    rows_h[32] = np.asarray(gamma, np.float32)
    rows_h[64] = np.asarray(beta, np.float32)
    cb_h = np.concatenate(
        [_tri_mask(), np.eye(P, dtype=np.float32).astype(BF16)], axis=1
    )
    bq = np.asarray(bq, np.float32)
    bk = np.asarray(bk, np.float32)

    # xT[p, b, kc, s] = x[b, s, kc*128+p]
    xT_h = np.ascontiguousarray(
        x.transpose(2, 0, 1).astype(BF16).reshape(8, P, 2, S).transpose(1, 2, 0, 3)
    )
    # selm[r, i, p] = 1 if r == 2i + (p>=64)
    selm_h = np.zeros((16, 8, P), np.float32)
    for i in range(8):
        selm_h[2 * i, i, 0:64] = 1.0
        selm_h[2 * i + 1, i, 64:P] = 1.0
    selm_h = selm_h.astype(BF16)

    in_maps = []
    for k in range(NCORES):
        hs = slice(k * P, (k + 1) * P)
        cf_h = np.zeros((P, 50), np.float32)
        cf_h[:, 0] = bq[hs]
        cf_h[:, 1] = bk[hs]
        cf_h[:, 2:10] = bv_h
        cf_h[:, 10:18] = bo_h
        cf_h[:, 18:50] = b1_h
        in_maps.append(
            {
                "xT": xT_h,
                "cb": cb_h,
                "cf": cf_h,
                "selm": selm_h,
                "wq": chunkT(Wq[:, hs], 8),
                "wk": chunkT(Wk[:, hs], 8),
                "wv": chunkT(Wv[:, hs], 8),
                "wo": wo_h,
                "w1": w1_h,
                "w2": w2_h,
                "rows": rows_h,
            }
        )
    return in_maps


def kernel(**inputs):
    in_maps = _prep_in_maps(**inputs)
    nc = _get_nc()
    res = run_bass_kernel_spmd(nc, in_maps, core_ids=list(range(NCORES)))
    out = np.zeros((B, S, D), np.float32)
    for k in range(NCORES):
        o = res.results[k]["out"]  # [512, D]: batch0 tokens then batch1
        out[0, k * 2 * P : (k + 1) * 2 * P] = o[0 : 2 * P]
        out[1, k * 2 * P : (k + 1) * 2 * P] = o[2 * P : 4 * P]
    return out
